# revision 12
# baseline (speedup 1.0000x reference)
"""CLIP ViT FM2 Bass kernel builder for TRN2 (8-core data-parallel).

Layout: feature-major residual stream x_fm [768 -> 6x(128), T=788] per core.
T columns = 4 sequences x 197 tokens: seqs 0-1 = x-stream (masked imgs),
seqs 2-3 = x2-stream (prompted imgs).  Dense matmuls in float32r (1 cyc/row
at N>=256 even), attention matmuls in bf16.
"""

import numpy as np
import ml_dtypes
import concourse.bass as bass
import concourse.tile as tile
from concourse import bacc, mybir

F32R = mybir.dt.float32r
F32 = mybir.dt.float32
BF16 = mybir.dt.bfloat16
AF = mybir.ActivationFunctionType
OP = mybir.AluOpType

D, H, HD, S, L = 768, 12, 64, 197, 196
FF, OUT = 3072, 512
KD, KF = 6, 24          # k-tiles over D and FF
NSEQ, T, TC = 4, 788, 394
NLAYERS_FULL = 12
MASKING_BLOCK = 10
SILU_S = 1.702


def build(n_layers=NLAYERS_FULL, debug=False, repeat=1, skip=()):
    nc = bacc.Bacc("TRN2", target_bir_lowering=False, debug=False, num_devices=8)

    def din(name, shape, dt=F32R):
        return nc.dram_tensor(name, shape, dt, kind="ExternalInput").ap()

    patches = din("patches", [KD, 128, 4 * L])
    poscls = din("poscls", [KD, 128, T])
    convw = din("convw", [KD, 128, D])
    lnpre_gb = din("lnpre_gb", [2, 128, KD], F32)
    nl1 = max(n_layers, 1)
    wqk = din("wqk", [nl1 * 3, 128, 3072], BF16)
    bqk = din("bqk", [nl1, 128, 12], F32)
    wv = din("wv", [nl1, 128, 4608], BF16)
    bv = din("bv", [nl1, 1, D], BF16)
    wo = din("wo", [nl1 * 2, 128, 2304], BF16)
    bo = din("bo", [nl1, 128, KD], F32)
    wfc = din("wfc", [nl1 * KF, 128, D], BF16)
    bfc = din("bfc", [nl1, 128, KF], F32)
    wp = din("wp", [nl1 * KD, 128, FF], BF16)
    bp = din("bp", [nl1, 128, KD], F32)
    pmb2 = din("pmb2", [128, TC])
    maskcol = din("maskcol", [128, 4], F32)
    ones_r = din("ones_r", [128, 1])
    ones_bf = din("ones_bf", [128, 1], BF16)
    ones2 = din("ones2", [1, 2])
    eps_d = din("eps_d", [1, 1], F32)
    projw = din("projw", [KD, 128, OUT])
    projb = din("projb", [1, OUT])
    out_d = nc.dram_tensor("out", [2, OUT], F32, kind="ExternalOutput").ap()
    dbg = {}
    if debug:
        for nm, shp, dt in [("dbg_ln1", [KD, 128, T], BF16), ("dbg_q", [KD, 128, T], BF16),
                            ("dbg_k", [KD, 128, T], BF16), ("dbg_v", [8, 128, D], BF16),
                            ("dbg_o", [KD, 128, T], BF16), ("dbg_x1", [KD, 128, T], F32),
                            ("dbg_h", [KF, 128, TC], F32), ("dbg_x2", [KD, 128, T], F32)]:
            dbg[nm] = nc.dram_tensor(nm, shp, dt, kind="ExternalOutput").ap()

    with tile.TileContext(nc) as tc:
        with (
            tc.tile_pool(name="acts", bufs=1) as acts,
            tc.tile_pool(name="consts", bufs=1) as consts,
            tc.tile_pool(name="rows", bufs=2) as rows,
            tc.tile_pool(name="bcasts", bufs=2) as bcasts,
            tc.tile_pool(name="scratch", bufs=2) as scratch,
        ):
            # persistent activation tiles
            xf = [acts.tile([128, T], F32R, name=f"xf{i}") for i in range(KD)]
            lnf = [acts.tile([128, T], BF16, name=f"lnf{i}") for i in range(KD)]

            def alloc_qkv():
                q = [acts.tile([128, T], BF16, name=f"qbf{i}", tag=f"qbf{i}") for i in range(KD)]
                k = [acts.tile([128, T], BF16, name=f"kbf{i}", tag=f"kbf{i}") for i in range(KD)]
                v = [acts.tile([128, D], BF16, name=f"vbf{i}", tag=f"vbf{i}") for i in range(8)]
                return q, k, v

            def alloc_ofm():
                return [acts.tile([128, T], BF16, name=f"ofm{i}", tag=f"ofm{i}") for i in range(KD)]

            def alloc_hfm():
                # overlays qbf/kbf/vbf slots (dead during MLP) to save SBUF
                hh = []
                for i in range(KF):
                    if i < 6:
                        tg = f"qbf{i}"
                    elif i < 12:
                        tg = f"kbf{i - 6}"
                    elif i < 20:
                        tg = f"vbf{i - 12}"
                    else:
                        tg = f"ofm{i - 20}"
                    hh.append(acts.tile([128, T], BF16, name=f"hfm{i}", tag=tg))
                return hh

            onr = consts.tile([128, 1], F32R, name="onr")
            nc.sync.dma_start(onr[:], ones_r[:])
            onb = consts.tile([128, 1], BF16, name="onb")
            nc.sync.dma_start(onb[:], ones_bf[:])
            on2 = consts.tile([1, 2], F32R, name="on2")
            nc.sync.dma_start(on2[:], ones2[:])
            eps_t = consts.tile([1, 1], F32, name="eps_t")
            nc.sync.dma_start(eps_t[:], eps_d[:])
            pmb2_t = consts.tile([128, TC], F32R, name="pmb2_t")
            nc.sync.dma_start(pmb2_t[:], pmb2[:])
            mcol_t = consts.tile([128, 4], F32, name="mcol_t")
            nc.sync.dma_start(mcol_t[:], maskcol[:])
            lnpre_t = consts.tile([128, 2 * KD], F32, name="lnpre_t")
            nc.sync.dma_start(lnpre_t[:, 0:KD], lnpre_gb[0])
            nc.sync.dma_start(lnpre_t[:, KD : 2 * KD], lnpre_gb[1])

            def ln_phase(src, dst, ncols=T, pre_gb=False):
                """dst[i] = LN(src) over feature dim (g/b folded into next W
                unless pre_gb)."""
                nchunk = (ncols + TC - 1) // TC
                with tc.tile_pool(name="lnpsum", bufs=2, space="PSUM") as lpp:
                    for tcc in range(nchunk):
                        c0 = tcc * TC
                        cw = min(TC, ncols - c0)
                        sl = slice(c0, c0 + cw)
                        s1 = lpp.tile([1, cw], F32, name="s1")
                        s2 = lpp.tile([1, cw], F32, name="s2")
                        for i in range(KD):
                            x2t = scratch.tile([128, TC], F32R, name="x2t")
                            nc.vector.tensor_mul(x2t[:, 0:cw], src[i][:, sl], src[i][:, sl])
                            nc.tensor.matmul(s1[:], onr[:], src[i][:, sl],
                                             start=(i == 0), stop=(i == KD - 1))
                            nc.tensor.matmul(s2[:], onr[:], x2t[:, 0:cw],
                                             start=(i == 0), stop=(i == KD - 1))
                        mean = rows.tile([1, TC], F32, name="mean")
                        nc.scalar.activation(mean[:, 0:cw], s1[:], AF.Copy, scale=1.0 / D)
                        ex2 = rows.tile([1, TC], F32, name="ex2")
                        nc.scalar.activation(ex2[:, 0:cw], s2[:], AF.Copy, scale=1.0 / D)
                        m2 = rows.tile([1, TC], F32, name="m2")
                        nc.vector.tensor_mul(m2[:, 0:cw], mean[:, 0:cw], mean[:, 0:cw])
                        nc.vector.tensor_sub(ex2[:, 0:cw], ex2[:, 0:cw], m2[:, 0:cw])
                        sd = rows.tile([1, TC], F32, name="sd")
                        nc.scalar.activation(sd[:, 0:cw], ex2[:, 0:cw], AF.Sqrt, bias=eps_t[:])
                        a_r = rows.tile([1, TC], F32, name="a_r")
                        nc.vector.reciprocal(a_r[:, 0:cw], sd[:, 0:cw])
                        c_r = rows.tile([1, TC], F32, name="c_r")
                        nc.vector.tensor_mul(c_r[:, 0:cw], mean[:, 0:cw], a_r[:, 0:cw])
                        nc.scalar.mul(c_r[:, 0:cw], c_r[:, 0:cw], -1.0)
                        a_b = bcasts.tile([128, TC], F32, name="a_b")
                        nc.gpsimd.partition_broadcast(a_b[:, 0:cw], a_r[:, 0:cw])
                        c_b = bcasts.tile([128, TC], F32, name="c_b")
                        nc.gpsimd.partition_broadcast(c_b[:, 0:cw], c_r[:, 0:cw])
                        for i in range(KD):
                            nc.vector.tensor_mul(dst[i][:, sl], src[i][:, sl],
                                                 a_b[:, 0:cw].bitcast(F32R))
                            nc.vector.tensor_add(dst[i][:, sl], dst[i][:, sl],
                                                 c_b[:, 0:cw].bitcast(F32R))
                            if pre_gb:
                                nc.vector.tensor_scalar(
                                    dst[i][:, sl], dst[i][:, sl],
                                    lnpre_t[:, i : i + 1], lnpre_t[:, KD + i : KD + i + 1],
                                    OP.mult, OP.add)

            # ---------------- embed ----------------
            with (
                tc.tile_pool(name="embw", bufs=2) as embw,
                tc.tile_pool(name="embpat", bufs=1) as embpat,
                tc.tile_pool(name="embps", bufs=2, space="PSUM") as embps,
            ):
                for i in range(KD):
                    nc.sync.dma_start(xf[i][:], poscls[i])
                pat = [embpat.tile([128, 4 * L], F32R, name=f"pat{i}") for i in range(KD)]
                for i in range(KD):
                    nc.sync.dma_start(pat[i][:], patches[i])
                for m in range(KD):
                    cwt = embw.tile([128, D], F32R, name="cwt")
                    nc.sync.dma_start(cwt[:], convw[m])
                    for tcc in range(2):
                        pp = embps.tile([128, 2 * L], F32, name="cvp")
                        for k in range(KD):
                            nc.tensor.matmul(pp[:], cwt[:, k * 128 : (k + 1) * 128],
                                             pat[k][:, tcc * 2 * L : (tcc + 1) * 2 * L],
                                             start=(k == 0), stop=(k == KD - 1))
                        for half in range(2):
                            sq = 2 * tcc + half
                            dst = xf[m][:, sq * S + 1 : sq * S + S]
                            nc.vector.tensor_add(dst, pp[:, half * L : (half + 1) * L], dst)
                ln_phase(xf, xf, pre_gb=True)

            # ---------------- layers ----------------
            for lyr in [ll for _ in range(repeat) for ll in range(n_layers)]:
                if lyr >= MASKING_BLOCK:
                    # x-stream input mix: x = 2*masked(x2) + x
                    for i in range(KD):
                        tmp = scratch.tile([128, TC], F32R, name="mixtmp")
                        nc.vector.tensor_mul(tmp[:], xf[i][:, TC:T], pmb2_t[:])
                        nc.vector.tensor_add(xf[i][:, 0:TC], xf[i][:, 0:TC], tmp[:])

                if "ln1" not in skip:
                    ln_phase(xf, lnf)
                if debug and lyr == 0:
                    for i in range(KD):
                        nc.sync.dma_start(dbg["dbg_ln1"][i], lnf[i][:])

                # ---- QKV ----
                qbf, kbf, vbf = alloc_qkv()
                if "qkv" in skip:
                    continue
                with (
                    tc.tile_pool(name="qkvw", bufs=2) as qkvw,
                    tc.tile_pool(name="qkvb", bufs=1) as qkvb,
                    tc.tile_pool(name="qkvps", bufs=2, space="PSUM") as qkvps,
                ):
                    bqk_sb = qkvb.tile([128, 12], F32, name="bqk_sb")
                    nc.sync.dma_start(bqk_sb[:], bqk[lyr])
                    for mg in range(3):
                        wt = qkvw.tile([128, 3072], BF16, name="wqk_t")
                        nc.sync.dma_start(wt[:], wqk[lyr * 3 + mg])
                        for tcc in range(2):
                            sl = slice(tcc * TC, (tcc + 1) * TC)
                            ps = [qkvps.tile([128, TC], F32, name=f"qkp{j}") for j in range(4)]
                            for k in range(KD):
                                for j in range(4):
                                    nc.tensor.matmul(
                                        ps[j][:], wt[:, (k * 4 + j) * 128 : (k * 4 + j + 1) * 128],
                                        lnf[k][:, sl], start=(k == 0), stop=(k == KD - 1))
                            for j in range(4):
                                m = mg * 4 + j
                                dest = qbf[m] if m < KD else kbf[m - KD]
                                nc.scalar.activation(dest[:, sl], ps[j][:], AF.Identity,
                                                     bias=bqk_sb[:, m : m + 1])
                    # ---- V (token-major, mode 1) ----
                    wvt = qkvb.tile([128, 4608], BF16, name="wvt")
                    nc.sync.dma_start(wvt[:], wv[lyr])
                    bvrow = qkvb.tile([1, D], BF16, name="bvrow")
                    nc.sync.dma_start(bvrow[:], bv[lyr])
                    bvb = qkvb.tile([128, D], BF16, name="bvb")
                    nc.gpsimd.partition_broadcast(bvb[:], bvrow[:])
                    for mt in range(8):
                        sq, half = mt // 2, mt % 2
                        tok0 = sq * S + half * 128
                        M = 128 if half == 0 else 69
                        for nc2 in range(2):
                            pv = qkvps.tile([128, TC], F32, name=f"qkp{nc2}")
                            for k in range(KD):
                                nc.tensor.matmul(
                                    pv[0:M, 0:384], lnf[k][:, tok0 : tok0 + M],
                                    wvt[:, (nc2 * KD + k) * 384 : (nc2 * KD + k + 1) * 384],
                                    start=(k == 0), stop=(k == KD - 1))
                            nc.vector.tensor_add(
                                vbf[mt][0:M, nc2 * 384 : (nc2 + 1) * 384], pv[0:M, 0:384],
                                bvb[0:M, nc2 * 384 : (nc2 + 1) * 384])

                if debug and lyr == 0:
                    for i in range(KD):
                        nc.sync.dma_start(dbg["dbg_q"][i], qbf[i][:])
                        nc.sync.dma_start(dbg["dbg_k"][i], kbf[i][:])
                    for i in range(8):
                        nc.sync.dma_start(dbg["dbg_v"][i], vbf[i][:])

                # ---- attention ----
                ofm = alloc_ofm()
                if "attn" in skip:
                    continue
                with (
                    tc.tile_pool(name="apool", bufs=2) as apool,
                    tc.tile_pool(name="sps", bufs=2, space="PSUM") as sps,
                    tc.tile_pool(name="csps", bufs=2, space="PSUM") as csps,
                    tc.tile_pool(name="avps", bufs=2, space="PSUM") as avps,
                ):
                    for hp in range(KD):
                        for sq in range(NSEQ):
                            base = sq * S
                            masked = (lyr >= MASKING_BLOCK) and sq >= 2
                            A = [apool.tile([128, 2 * S], BF16, name=f"at{hh}") for hh in range(2)]
                            for hh in range(2):
                                po = hh * 64
                                sp0 = sps.tile([128, S], F32, name="sp0")
                                sp1 = sps.tile([128, S], F32, name="sp1")
                                nc.tensor.matmul(sp0[:], kbf[hp][po : po + 64, base : base + 128],
                                                 qbf[hp][po : po + 64, base : base + S],
                                                 start=True, stop=True)
                                nc.tensor.matmul(sp1[0:69, :],
                                                 kbf[hp][po : po + 64, base + 128 : base + S],
                                                 qbf[hp][po : po + 64, base : base + S],
                                                 start=True, stop=True)
                                nc.scalar.activation(A[hh][:, 0:S], sp0[:], AF.Exp, scale=0.125)
                                nc.scalar.activation(A[hh][0:69, S : 2 * S], sp1[0:69, :],
                                                     AF.Exp, scale=0.125)
                                if masked:
                                    mc = (sq - 2) * 2
                                    nc.vector.tensor_scalar(
                                        A[hh][:, 0:1], A[hh][:, 0:1],
                                        mcol_t[:, mc : mc + 1], None, OP.mult)
                                    nc.vector.tensor_scalar(
                                        A[hh][0:69, S : S + 1], A[hh][0:69, S : S + 1],
                                        mcol_t[0:69, mc + 1 : mc + 2], None, OP.mult)
                            cs = csps.tile([128, S], F32, name="cs")
                            for hh in range(2):
                                r = hh * 64
                                nc.tensor.matmul(cs[r : r + 1, :], onb[:], A[hh][:, 0:S],
                                                 start=True, stop=False)
                                nc.tensor.matmul(cs[r : r + 1, :], onb[0:69, :],
                                                 A[hh][0:69, S : 2 * S],
                                                 start=False, stop=True)
                            recbs = []
                            for hh in range(2):
                                rec = rows.tile([1, S], F32, name=f"rec{hh}")
                                nc.vector.reciprocal(rec[:], cs[hh * 64 : hh * 64 + 1, :])
                                recb = bcasts.tile([128, S], F32, name=f"recb{hh}")
                                nc.gpsimd.partition_broadcast(recb[:], rec[:])
                                recbs.append(recb)
                            av = avps.tile([128, S], F32, name="av")
                            for hh in range(2):
                                h = 2 * hp + hh
                                po = hh * 64
                                nc.tensor.matmul(av[po : po + 64, :],
                                                 vbf[2 * sq][:, h * 64 : h * 64 + 64],
                                                 A[hh][:, 0:S], start=True, stop=False)
                                nc.tensor.matmul(av[po : po + 64, :],
                                                 vbf[2 * sq + 1][0:69, h * 64 : h * 64 + 64],
                                                 A[hh][0:69, S : 2 * S], start=False, stop=True)
                            for hh in range(2):
                                po = hh * 64
                                nc.vector.tensor_mul(
                                    ofm[hp][po : po + 64, base : base + S],
                                    av[po : po + 64, :],
                                    recbs[hh][po : po + 64, :].bitcast(F32R))

                if debug and lyr == 0:
                    for i in range(KD):
                        nc.sync.dma_start(dbg["dbg_o"][i], ofm[i][:])

                # ---- Wo + residual ----
                if "wo" in skip:
                    continue
                with (
                    tc.tile_pool(name="wow", bufs=2) as wow,
                    tc.tile_pool(name="wob", bufs=1) as wob,
                    tc.tile_pool(name="wops", bufs=2, space="PSUM") as wops,
                ):
                    bo_sb = wob.tile([128, KD], F32, name="bo_sb")
                    nc.sync.dma_start(bo_sb[:], bo[lyr])
                    for mgrp in range(2):
                        wt = wow.tile([128, 2304], BF16, name="wo_t")
                        nc.sync.dma_start(wt[:], wo[lyr * 2 + mgrp])
                        for tcc in range(2):
                            sl = slice(tcc * TC, (tcc + 1) * TC)
                            ps = [wops.tile([128, TC], F32, name=f"wop{j}") for j in range(3)]
                            for k in range(KD):
                                for j in range(3):
                                    nc.tensor.matmul(
                                        ps[j][:], wt[:, (k * 3 + j) * 128 : (k * 3 + j + 1) * 128],
                                        ofm[k][:, sl], start=(k == 0), stop=(k == KD - 1))
                            for j in range(3):
                                m = mgrp * 3 + j
                                nc.vector.scalar_tensor_tensor(
                                    xf[m][:, sl], ps[j][:], bo_sb[:, m : m + 1],
                                    xf[m][:, sl], OP.add, OP.add)

                if debug and lyr == 0:
                    for i in range(KD):
                        nc.sync.dma_start(dbg["dbg_x1"][i], xf[i][:].bitcast(F32))

                # ---- MLP ----
                if "mlp" in skip:
                    continue
                if "ln2" not in skip:
                    ln_phase(xf, lnf)
                hfm = alloc_hfm()
                with (
                    tc.tile_pool(name="fcw", bufs=3) as fcw,
                    tc.tile_pool(name="fcb", bufs=1) as fcb,
                    tc.tile_pool(name="fc1ps", bufs=4, space="PSUM") as fc1ps,
                    tc.tile_pool(name="fc2ps", bufs=4, space="PSUM") as fc2ps,
                ):
                    bfc_sb = fcb.tile([128, KF], F32, name="bfc_sb")
                    nc.sync.dma_start(bfc_sb[:], bfc[lyr])
                    bp_sb = fcb.tile([128, KD], F32, name="bp_sb")
                    nc.sync.dma_start(bp_sb[:], bp[lyr])
                    for mg in range(KF):
                        w1 = fcw.tile([128, D], BF16, name="wfc_t")
                        nc.sync.dma_start(w1[:], wfc[lyr * KF + mg])
                        for tcc in range(2):
                            sl = slice(tcc * TC, (tcc + 1) * TC)
                            p1 = fc1ps.tile([128, TC], F32, name="fc1p")
                            for k in range(KD):
                                nc.tensor.matmul(p1[:], w1[:, k * 128 : (k + 1) * 128],
                                                 lnf[k][:, sl], start=(k == 0), stop=(k == KD - 1))
                            nc.scalar.activation(hfm[mg][:, sl], p1[:], AF.Silu,
                                                 bias=bfc_sb[:, mg : mg + 1], scale=SILU_S)
                    for m2 in range(KD):
                        w2 = fcw.tile([128, FF], BF16, name="wp_t")
                        nc.sync.dma_start(w2[:], wp[lyr * KD + m2])
                        for tcc in range(2):
                            sl = slice(tcc * TC, (tcc + 1) * TC)
                            p2 = fc2ps.tile([128, TC], F32, name="fc2p")
                            for k in range(KF):
                                nc.tensor.matmul(p2[:], w2[:, k * 128 : (k + 1) * 128],
                                                 hfm[k][:, sl], start=(k == 0), stop=(k == KF - 1))
                            nc.vector.scalar_tensor_tensor(
                                xf[m2][:, sl], p2[:], bp_sb[:, m2 : m2 + 1],
                                xf[m2][:, sl], OP.add, OP.add)

            if debug:
                for i in range(KD):
                    nc.sync.dma_start(dbg["dbg_x2"][i], xf[i][:].bitcast(F32))

            # ---------------- final ----------------
            with (
                tc.tile_pool(name="finpool", bufs=1) as finpool,
                tc.tile_pool(name="finps", bufs=1, space="PSUM") as finps,
            ):
                cls_sb = finpool.tile([128, 2 * KD], F32R, name="cls_sb")
                for i in range(KD):
                    nc.vector.tensor_copy(cls_sb[:, 2 * i : 2 * i + 2],
                                          xf[i].rearrange("p (s t) -> p s t", s=NSEQ)[:, 0:2, 0])
                # ln_post stats over the 2 cls tokens
                s1 = finps.tile([1, 2], F32, name="fs1")
                s2 = finps.tile([1, 2], F32, name="fs2")
                for i in range(KD):
                    x2t = scratch.tile([128, TC], F32R, name="x2t")
                    nc.vector.tensor_mul(x2t[:, 0:2], cls_sb[:, 2 * i : 2 * i + 2],
                                         cls_sb[:, 2 * i : 2 * i + 2])
                    nc.tensor.matmul(s1[:], onr[:], cls_sb[:, 2 * i : 2 * i + 2],
                                     start=(i == 0), stop=(i == KD - 1))
                    nc.tensor.matmul(s2[:], onr[:], x2t[:, 0:2],
                                     start=(i == 0), stop=(i == KD - 1))
                mean = rows.tile([1, TC], F32, name="mean")
                nc.scalar.activation(mean[:, 0:2], s1[:], AF.Copy, scale=1.0 / D)
                ex2 = rows.tile([1, TC], F32, name="ex2")
                nc.scalar.activation(ex2[:, 0:2], s2[:], AF.Copy, scale=1.0 / D)
                m2r = rows.tile([1, TC], F32, name="m2")
                nc.vector.tensor_mul(m2r[:, 0:2], mean[:, 0:2], mean[:, 0:2])
                nc.vector.tensor_sub(ex2[:, 0:2], ex2[:, 0:2], m2r[:, 0:2])
                sd = rows.tile([1, TC], F32, name="sd")
                nc.scalar.activation(sd[:, 0:2], ex2[:, 0:2], AF.Sqrt, bias=eps_t[:])
                a_r = rows.tile([1, TC], F32, name="a_r")
                nc.vector.reciprocal(a_r[:, 0:2], sd[:, 0:2])
                c_r = rows.tile([1, TC], F32, name="c_r")
                nc.vector.tensor_mul(c_r[:, 0:2], mean[:, 0:2], a_r[:, 0:2])
                nc.scalar.mul(c_r[:, 0:2], c_r[:, 0:2], -1.0)
                a_b = bcasts.tile([128, TC], F32, name="a_b")
                nc.gpsimd.partition_broadcast(a_b[:, 0:2], a_r[:, 0:2])
                c_b = bcasts.tile([128, TC], F32, name="c_b")
                nc.gpsimd.partition_broadcast(c_b[:, 0:2], c_r[:, 0:2])
                clsln = finpool.tile([128, 2 * KD], F32R, name="clsln")
                for i in range(KD):
                    nc.vector.tensor_mul(clsln[:, 2 * i : 2 * i + 2],
                                         cls_sb[:, 2 * i : 2 * i + 2],
                                         a_b[:, 0:2].bitcast(F32R))
                    nc.vector.tensor_add(clsln[:, 2 * i : 2 * i + 2],
                                         clsln[:, 2 * i : 2 * i + 2],
                                         c_b[:, 0:2].bitcast(F32R))
                # proj: out[2, 512] = clsln.T @ proj' + projb
                pw = [finpool.tile([128, OUT], F32R, name=f"pw{k}") for k in range(KD)]
                for k in range(KD):
                    nc.sync.dma_start(pw[k][:], projw[k])
                pb = finpool.tile([1, OUT], F32R, name="pb")
                nc.sync.dma_start(pb[:], projb[:])
                pp = finps.tile([2, OUT], F32, name="pp")
                for k in range(KD):
                    nc.tensor.matmul(pp[:], clsln[:, 2 * k : 2 * k + 2], pw[k][:],
                                     start=(k == 0), stop=False)
                nc.tensor.matmul(pp[:], on2[:], pb[:], start=False, stop=True)
                out_sb = finpool.tile([2, OUT], F32, name="out_sb")
                nc.scalar.activation(out_sb[:], pp[:], AF.Copy)
                nc.sync.dma_start(out_d[:], out_sb[:])

    nc.compile()
    return nc


# ===================== host-side prep =====================

def _img_patches_fm(img):
    """[3,224,224] -> feature-major [768, 196] matching conv_w.reshape(768,768)."""
    p = img.reshape(3, 14, 16, 14, 16).transpose(1, 3, 0, 2, 4).reshape(196, 768)
    return np.ascontiguousarray(p.T)


def prep_core(inputs, c, n_layers=NLAYERS_FULL):
    f32 = np.float32
    bf16 = ml_dtypes.bfloat16
    m = {}
    imgs = [inputs["masked_img"][2 * c], inputs["masked_img"][2 * c + 1],
            inputs["prompted_img"][2 * c], inputs["prompted_img"][2 * c + 1]]
    pat = np.concatenate([_img_patches_fm(np.asarray(im, f32)) for im in imgs], axis=1)  # [768, 784]
    m["patches"] = pat.reshape(KD, 128, 4 * L).astype(f32)

    class_emb = np.asarray(inputs["class_emb"], f32)
    pos_emb = np.asarray(inputs["pos_emb"], f32)  # [197, 768]
    blk = pos_emb.T.copy()  # [768, 197]
    blk[:, 0] += class_emb
    m["poscls"] = np.tile(blk, (1, NSEQ)).reshape(KD, 128, T).astype(f32)

    convWT = np.asarray(inputs["conv_w"], f32).reshape(D, D).T  # [in, out]
    m["convw"] = np.stack([
        convWT[:, mm * 128:(mm + 1) * 128].reshape(KD, 128, 128)
        .transpose(1, 0, 2).reshape(128, D)
        for mm in range(KD)])
    m["lnpre_gb"] = np.stack([
        np.asarray(inputs["ln_pre_g"], f32).reshape(KD, 128).T,
        np.asarray(inputs["ln_pre_b"], f32).reshape(KD, 128).T])

    Wqkv = np.asarray(inputs["Wqkv"], f32)
    bqkv = np.asarray(inputs["bqkv"], f32)
    g1 = np.asarray(inputs["ln1_g"], f32)
    b1 = np.asarray(inputs["ln1_b"], f32)
    g2 = np.asarray(inputs["ln2_g"], f32)
    b2 = np.asarray(inputs["ln2_b"], f32)
    Wo = np.asarray(inputs["Wo"], f32)
    bo_ = np.asarray(inputs["bo"], f32)
    Wfc = np.asarray(inputs["Wfc"], f32)
    bfc_ = np.asarray(inputs["bfc"], f32)
    Wp = np.asarray(inputs["Wp"], f32)
    bp_ = np.asarray(inputs["bp"], f32)

    wqk_l, bqk_l, wv_l, bv_l, wo_l, bo_l = [], [], [], [], [], []
    wfc_l, bfc_l, wp_l, bp_l = [], [], [], []
    for l in range(n_layers):
        Wq = Wqkv[l] * g1[l][None, :]
        bq = bqkv[l] + Wqkv[l] @ b1[l]
        WT = Wq.T.copy()  # [768, 2304]
        qk = WT[:, :1536].reshape(KD, 128, 12, 128)
        for mg in range(3):
            wqk_l.append(qk[:, :, mg * 4:(mg + 1) * 4, :].transpose(1, 0, 2, 3).reshape(128, 3072))
        bqk_l.append(bq[:1536].reshape(12, 128).T)
        vp = WT[:, 1536:2304].reshape(KD, 128, 2, 384)
        wv_l.append(vp.transpose(1, 2, 0, 3).reshape(128, 4608))
        bv_l.append(bq[1536:2304].reshape(1, D))

        WoT = Wo[l].T.reshape(KD, 128, KD, 128)
        for mgrp in range(2):
            wo_l.append(WoT[:, :, mgrp * 3:(mgrp + 1) * 3, :].transpose(1, 0, 2, 3).reshape(128, 2304))
        bo_l.append(bo_[l].reshape(KD, 128).T)

        Wf = Wfc[l] * g2[l][None, :]
        bf = bfc_[l] + Wfc[l] @ b2[l]
        WfT = Wf.T.reshape(KD, 128, KF, 128)
        for mg in range(KF):
            wfc_l.append(WfT[:, :, mg, :].transpose(1, 0, 2).reshape(128, D))
        bfc_l.append((SILU_S * bf).reshape(KF, 128).T)

        WpT = (Wp[l].T / SILU_S).reshape(KF, 128, KD, 128)
        for m2 in range(KD):
            wp_l.append(WpT[:, :, m2, :].transpose(1, 0, 2).reshape(128, FF))
        bp_l.append(bp_[l].reshape(KD, 128).T)

    if n_layers == 0:
        wqk_l = [np.zeros((128, 3072))] * 3
        bqk_l = [np.zeros((128, 12))]
        wv_l = [np.zeros((128, 4608))]
        bv_l = [np.zeros((1, D))]
        wo_l = [np.zeros((128, 2304))] * 2
        bo_l = [np.zeros((128, KD))]
        wfc_l = [np.zeros((128, D))] * KF
        bfc_l = [np.zeros((128, KF))]
        wp_l = [np.zeros((128, FF))] * KD
        bp_l = [np.zeros((128, KD))]
    m["wqk"] = np.stack(wqk_l).astype(bf16)
    m["bqk"] = np.stack(bqk_l).astype(f32)
    m["wv"] = np.stack(wv_l).astype(bf16)
    m["bv"] = np.stack(bv_l).astype(bf16)
    m["wo"] = np.stack(wo_l).astype(bf16)
    m["bo"] = np.stack(bo_l).astype(f32)
    m["wfc"] = np.stack(wfc_l).astype(bf16)
    m["bfc"] = np.stack(bfc_l).astype(f32)
    m["wp"] = np.stack(wp_l).astype(bf16)
    m["bp"] = np.stack(bp_l).astype(f32)

    pm = [np.asarray(inputs["pred_masks"][2 * c + s], f32).reshape(L) for s in range(2)]
    pmb = np.empty((1, TC), f32)
    for s in range(2):
        pmb[0, s * S] = 2.0
        pmb[0, s * S + 1: s * S + S] = 2.0 * pm[s]
    m["pmb2"] = np.broadcast_to(pmb, (128, TC)).copy()

    mcol = np.ones((128, 4), f32)
    for s in range(2):
        mcol[1:128, s * 2] = pm[s][0:127]
        mcol[0:69, s * 2 + 1] = pm[s][127:196]
    m["maskcol"] = mcol.astype(f32)

    m["ones_r"] = np.ones((128, 1), f32)
    m["ones_bf"] = np.ones((128, 1), bf16)
    m["ones2"] = np.ones((1, 2), f32)
    m["eps_d"] = np.full((1, 1), 1e-5, f32)

    gp = np.asarray(inputs["ln_post_g"], f32)
    bpost = np.asarray(inputs["ln_post_b"], f32)
    proj = np.asarray(inputs["proj"], f32)
    projp = gp[:, None] * proj
    m["projw"] = projp.reshape(KD, 128, OUT).astype(f32)
    m["projb"] = (bpost @ proj).reshape(1, OUT).astype(f32)
    return m


# ===================== kernel entry =====================

_CACHE = {}


def _get_nc():
    if "nc" not in _CACHE:
        _CACHE["nc"] = build(NLAYERS_FULL)
    return _CACHE["nc"]


def kernel(**inputs):
    """Full-input entry: shards batch N=16 across 8 NeuronCores (2 samples
    per core; each core runs both streams for its samples), returns [16, 512]."""
    from concourse.bass_utils import run_bass_kernel_spmd

    nc = _get_nc()
    inputs = {k: np.asarray(v) for k, v in inputs.items()}
    in_maps = [prep_core(inputs, c, NLAYERS_FULL) for c in range(8)]
    res = run_bass_kernel_spmd(nc, in_maps, core_ids=list(range(8)))
    out = np.concatenate([res.results[c]["out"] for c in range(8)], axis=0)
    return out.astype(np.float32)

